# revision 7
# baseline (speedup 1.0000x reference)
"""Trainium2 Bass kernel for nn_Pool_conv_sum_nonlin_pool (B=4,C=16,O=32,P=256,G=2048).

Strategy: shard the P axis across 8 NeuronCores (32 rows each). Per core:
  phase 1: stream the x_pg shard into SBUF (resident, 16.8MB), computing
           pool_g (max over G, local) on VectorE and the local partial of
           pool_p (max over P) split across VectorE/GpSimd.
  AllReduce(max) of the pool_p partial (512KB) across the 8 cores.
  y_p / y_g are computed on TensorE with block-diagonal stacked weights
  (partition layout (h,b,c) for inputs, (b,o) for outputs; B*O = 128).
  phase 2: per p-row, one PSUM accumulation per 512-wide g-chunk:
           identity-matmul adds y_g, the main matmul adds W_pg_pg @ x;
           ScalarE applies Lrelu(t + y_p[b,o,p] + biases) in one op;
           VectorE reduces out_p, VectorE+GpSimd keep the out_g running max;
           1MB HWDGE DMA per p-row streams out_pg back to HBM.
Host: concatenates out_pg / out_p shards, max-combines the 8 out_g partials.
"""
import sys

sys.path.insert(0, "/opt/trn_rl_repo")

import numpy as np

import concourse.bacc as bacc
import concourse.mybir as mybir
from concourse import tile
from concourse.bass_utils import run_bass_kernel_spmd

B, C, O, P, G = 4, 16, 32, 256, 2048
N_CORES = 8
P_LOC = P // N_CORES          # 32
H = G // 2                    # 1024 (partition-half split of g)
NEG_SLOPE = 0.01
NEG_INF = -3.0e38

# NOTE: this neuronxcc build rejects compute ops on the GpSimd/Pool engine
# ("Engine Pool is invalid ... Must be DVE"), so all max/reduce work runs on
# VectorE and the fused bias+Lrelu on ScalarE.

_CACHE = {}


def _stack_weight(w: np.ndarray) -> np.ndarray:
    """[O,C] conv weight -> [128,128] lhsT with block-diag over b, duplicated
    into both partition halves (rows 64h+16b+c, cols 32b+o)."""
    s = np.zeros((64, 128), dtype=np.float32)
    blk = np.ascontiguousarray(w.T.astype(np.float32))  # [C, O]
    for b in range(B):
        s[16 * b:16 * b + 16, 32 * b:32 * b + 32] = blk
    return np.concatenate([s, s], axis=0)


def _build(consts):
    nc = bacc.Bacc("TRN2", target_bir_lowering=False, debug=False,
                   num_devices=N_CORES)
    f32 = mybir.dt.float32

    x = nc.declare_dram_parameter("x", [B, C, P_LOC, G], f32, isOutput=False)
    xp = nc.declare_dram_parameter("xp", [B, C, P_LOC], f32, isOutput=False)
    xg = nc.declare_dram_parameter("xg", [B, C, G], f32, isOutput=False)
    out_pg = nc.declare_dram_parameter("out_pg", [B, O, P_LOC, G], f32, isOutput=True)
    out_p = nc.declare_dram_parameter("out_p", [B, O, P_LOC], f32, isOutput=True)
    out_gp = nc.declare_dram_parameter("out_gp", [B, O, G], f32, isOutput=True)

    with tile.TileContext(nc) as tc:
        with (
            tc.tile_pool(name="const", bufs=1) as cpool,
            tc.tile_pool(name="xtiles", bufs=P_LOC) as xpool,
            tc.tile_pool(name="acc", bufs=1) as apool,
            tc.tile_pool(name="outs", bufs=3) as opool,
            tc.tile_pool(name="psum", bufs=2, space="PSUM") as ppool,
            tc.tile_pool(name="dram", bufs=1, space="DRAM") as dpool,
        ):
            # ---- constants -------------------------------------------------
            sb = {}
            for name in ("w_pgpg", "w_pgg", "w_g", "w_pgp", "w_p", "ident"):
                t = cpool.tile([128, 128], f32, tag=name)
                nc.sync.dma_start(t[:], nc.inline_tensor(consts[name], name)[:, :])
                sb[name] = t
            for name in ("bias_yg", "bias_ypb"):
                t = cpool.tile([128, 1], f32, tag=name)
                nc.sync.dma_start(t[:], nc.inline_tensor(consts[name], name)[:, :])
                sb[name] = t

            # ---- small inputs ---------------------------------------------
            xg_sb = cpool.tile([128, H], f32, tag="xg_sb")
            nc.sync.dma_start(xg_sb[:], xg[:, :, :].rearrange("b c (h g) -> h b c g", h=2))
            xp_sb = cpool.tile([64, P_LOC], f32, tag="xp_sb")
            nc.sync.dma_start(xp_sb[:], xp[:, :, :].rearrange("b c p -> (b c) p"))

            # ---- accumulators ----------------------------------------------
            accp = apool.tile([128, H], f32, tag="accp")
            accg = apool.tile([128, G], f32, tag="accg")
            pgc = apool.tile([128, P_LOC], f32, tag="pgc")
            outp_sb = apool.tile([128, P_LOC], f32, tag="outp_sb")
            nc.vector.memset(accp[:], NEG_INF)
            nc.vector.memset(accg[:], NEG_INF)

            # ---- phase 1: load x shard, compute pools ----------------------
            xt = []
            for p in range(P_LOC):
                t = xpool.tile([128, H], f32, tag="xt")
                nc.sync.dma_start(t[:], x[:, :, p, :].rearrange("b c (h g) -> h b c g", h=2))
                xt.append(t)
                nc.vector.tensor_tensor(accp[:], accp[:], t[:, :],
                                        op=mybir.AluOpType.max)
                nc.vector.reduce_max(pgc[:, p:p + 1], t[:, :],
                                     axis=mybir.AxisListType.X)

            # pool_g: combine the two g-halves (cross-partition via SBUF DMA)
            pgd = apool.tile([64, P_LOC], f32, tag="pgd")
            nc.sync.dma_start(pgd[:], pgc[64:128, :])
            pgf = apool.tile([64, P_LOC], f32, tag="pgf")
            nc.vector.tensor_tensor(pgf[:], pgc[0:64, :], pgd[:],
                                    op=mybir.AluOpType.max)

            # ---- AllReduce(max) of partial pool_p --------------------------
            bounce_in = dpool.tile([128, H], f32, tag="bin")
            bounce_out = dpool.tile([128, H], f32, tag="bout")
            nc.sync.dma_start(bounce_in[:, :], accp[:])
            nc.gpsimd.collective_compute(
                "AllReduce", mybir.AluOpType.max,
                replica_groups=[list(range(N_CORES))],
                ins=[bounce_in.opt()], outs=[bounce_out.opt()],
            )
            pool_sb = cpool.tile([128, H], f32, tag="pool_sb")
            nc.sync.dma_start(pool_sb[:], bounce_out[:])

            # ---- y_p = W_pgp @ pool_g + W_p @ xp (+biases via ACT) ---------
            ps_yp = ppool.tile([128, P_LOC], f32, tag="ps")
            nc.tensor.matmul(ps_yp[:], sb["w_pgp"][0:64, :], pgf[:],
                             start=True, stop=False)
            nc.tensor.matmul(ps_yp[:], sb["w_p"][0:64, :], xp_sb[:],
                             start=False, stop=True)
            ypb = apool.tile([128, P_LOC], f32, tag="ypb")
            nc.scalar.activation(ypb[:], ps_yp[:],
                                 mybir.ActivationFunctionType.Identity,
                                 bias=sb["bias_ypb"][:, 0:1], scale=1.0)

            # ---- y_g = W_pgg @ pool_p + W_g @ xg (+biases via ACT) ---------
            yg_sb = cpool.tile([128, G], f32, tag="yg_sb")
            for q in range(4):
                h, sl = divmod(q, 2)
                lo, hi = 64 * h, 64 * h + 64
                gs = slice(512 * sl, 512 * sl + 512)
                ps = ppool.tile([128, 512], f32, tag="ps")
                nc.tensor.matmul(ps[:], sb["w_pgg"][lo:hi, :], pool_sb[lo:hi, gs],
                                 start=True, stop=False)
                nc.tensor.matmul(ps[:], sb["w_g"][lo:hi, :], xg_sb[lo:hi, gs],
                                 start=False, stop=True)
                nc.scalar.activation(yg_sb[:, 512 * q:512 * q + 512], ps[:],
                                     mybir.ActivationFunctionType.Identity,
                                     bias=sb["bias_yg"][:, 0:1], scale=1.0)

            # ---- phase 2: per p-row compute + pools + store ----------------
            for p in range(P_LOC):
                ps = ppool.tile([128, G], f32, tag="ps")
                for q in range(4):
                    h, sl = divmod(q, 2)
                    lo, hi = 64 * h, 64 * h + 64
                    cs = slice(512 * q, 512 * q + 512)
                    gs = slice(512 * sl, 512 * sl + 512)
                    nc.tensor.matmul(ps[:, cs], sb["ident"][:], yg_sb[:, cs],
                                     start=True, stop=False)
                    nc.tensor.matmul(ps[:, cs], sb["w_pgpg"][lo:hi, :],
                                     xt[p][lo:hi, gs], start=False, stop=True)
                ot = opool.tile([128, G], f32, tag="ot")
                for q in range(4):
                    cs = slice(512 * q, 512 * q + 512)
                    nc.scalar.activation(ot[:, cs], ps[:, cs],
                                         mybir.ActivationFunctionType.Lrelu,
                                         bias=ypb[:, p:p + 1], scale=1.0,
                                         alpha=NEG_SLOPE)
                nc.sync.dma_start(out_pg[:, :, p, :].rearrange("b o g -> (b o) g"),
                                  ot[:])
                nc.vector.reduce_max(outp_sb[:, p:p + 1], ot[:, :],
                                     axis=mybir.AxisListType.X)
                nc.vector.tensor_tensor(accg[:], accg[:], ot[:, :],
                                        op=mybir.AluOpType.max)

            # ---- small outputs ---------------------------------------------
            nc.sync.dma_start(out_p[:, :, :].rearrange("b o p -> (b o) p"),
                              outp_sb[:])
            nc.sync.dma_start(out_gp[:, :, :].rearrange("b o g -> (b o) g"),
                              accg[:])

    nc.compile()
    return nc


def _get_nc(consts):
    if "nc" not in _CACHE:
        _CACHE["nc"] = _build(consts)
    return _CACHE["nc"]


def kernel(x_pg, x_p, x_g,
           W_pg_p, b_pg_p, W_pg_g, b_pg_g, W_pg_pg, b_pg_pg,
           W_p, b_p, W_g, b_g, _trace=False):
    x_pg = np.asarray(x_pg, dtype=np.float32)
    x_p = np.asarray(x_p, dtype=np.float32)
    x_g = np.asarray(x_g, dtype=np.float32)

    def col(bias):
        return np.ascontiguousarray(
            np.tile(np.asarray(bias, np.float32), B)[:, None])

    consts = {
        "w_pgpg": _stack_weight(np.asarray(W_pg_pg, np.float32)),
        "w_pgg": _stack_weight(np.asarray(W_pg_g, np.float32)),
        "w_g": _stack_weight(np.asarray(W_g, np.float32)),
        "w_pgp": _stack_weight(np.asarray(W_pg_p, np.float32)),
        "w_p": _stack_weight(np.asarray(W_p, np.float32)),
        "ident": np.eye(128, dtype=np.float32),
        "bias_yg": col(np.asarray(b_pg_g, np.float32) + np.asarray(b_g, np.float32)),
        "bias_ypb": col(np.asarray(b_pg_p, np.float32) + np.asarray(b_p, np.float32)
                        + np.asarray(b_pg_pg, np.float32)),
    }
    nc = _get_nc(consts)

    x4 = x_pg[..., 0]                      # [B,C,P,G]
    xp3 = x_p[:, :, :, 0, 0]               # [B,C,P]
    xg3 = np.ascontiguousarray(x_g[:, :, 0, :, 0])  # [B,C,G]

    in_maps = []
    for i in range(N_CORES):
        sl = slice(P_LOC * i, P_LOC * (i + 1))
        in_maps.append({
            "x": np.ascontiguousarray(x4[:, :, sl, :]),
            "xp": np.ascontiguousarray(xp3[:, :, sl]),
            "xg": xg3,
        })

    res = run_bass_kernel_spmd(nc, in_maps, core_ids=list(range(N_CORES)),
                               trace=_trace)
    if _trace:
        _CACHE["last_result"] = res

    out_pg = np.concatenate([r["out_pg"] for r in res.results], axis=2)
    out_p = np.concatenate([r["out_p"] for r in res.results], axis=2)
    out_g = np.maximum.reduce([r["out_gp"] for r in res.results])

    return (out_pg[:, :, :, :, None],
            out_p[:, :, :, None, None],
            out_g[:, :, None, :, None])


# revision 35
# speedup vs baseline: 281.5268x; 281.5268x over previous
"""Trainium2 Bass kernel for nn_Pool_conv_sum_nonlin_pool (B=4,C=16,O=32,P=256,G=2048).

Strategy: shard the P axis across 8 NeuronCores (32 rows each). Per core:
  phase 1: stream the x_pg shard into SBUF (resident, 16.8MB), computing
           pool_g (max over G, local) on VectorE and the local partial of
           pool_p (max over P) split across VectorE/GpSimd.
  AllReduce(max) of the pool_p partial (512KB) across the 8 cores.
  y_p / y_g are computed on TensorE with block-diagonal stacked weights
  (partition layout (h,b,c) for inputs, (b,o) for outputs; B*O = 128).
  phase 2: per p-row, one PSUM accumulation per 512-wide g-chunk:
           identity-matmul adds y_g, the main matmul adds W_pg_pg @ x;
           ScalarE applies Lrelu(t + y_p[b,o,p] + biases) in one op;
           VectorE reduces out_p, VectorE+GpSimd keep the out_g running max;
           1MB HWDGE DMA per p-row streams out_pg back to HBM.
Host: concatenates out_pg / out_p shards, max-combines the 8 out_g partials.
"""
import sys

sys.path.insert(0, "/opt/trn_rl_repo")

import ml_dtypes
import numpy as np

import concourse.bacc as bacc
import concourse.mybir as mybir
from concourse import tile
from concourse.bass_utils import run_bass_kernel_spmd

B, C, O, P, G = 4, 16, 32, 256, 2048
N_CORES = 8
P_LOC = P // N_CORES          # 32
H = G // 2                    # 1024 (partition-half split of g)
NEG_SLOPE = 0.01
NEG_INF = -3.0e38

# NOTE: this neuronxcc build rejects compute ops on the GpSimd/Pool engine
# ("Engine Pool is invalid ... Must be DVE"), so all max/reduce work runs on
# VectorE and the fused bias+Lrelu on ScalarE.

_CACHE = {}


def _stack_weight(w: np.ndarray) -> np.ndarray:
    """[O,C] conv weight -> [128,128] lhsT with block-diag over b, duplicated
    into both partition halves (rows 64h+16b+c, cols 32b+o)."""
    s = np.zeros((64, 128), dtype=np.float32)
    blk = np.ascontiguousarray(w.T.astype(np.float32))  # [C, O]
    for b in range(B):
        s[16 * b:16 * b + 16, 32 * b:32 * b + 32] = blk
    return np.concatenate([s, s], axis=0)


def _build(consts, sim=False):
    # sim=True builds a single-core variant with the collective replaced by a
    # local DRAM copy, for TimelineSim cost-model runs (no collective support).
    nc = bacc.Bacc("TRN2", target_bir_lowering=False, debug=False,
                   num_devices=1 if sim else N_CORES)
    f32 = mybir.dt.float32

    x = nc.declare_dram_parameter("x", [B, C, P_LOC, G], f32, isOutput=False)
    xp = nc.declare_dram_parameter("xp", [B, C, P_LOC], f32, isOutput=False)
    xg = nc.declare_dram_parameter("xg", [B, C, G], f32, isOutput=False)
    out_pg = nc.declare_dram_parameter("out_pg", [B, O, P_LOC, G], f32, isOutput=True)
    out_p = nc.declare_dram_parameter("out_p", [B, O, P_LOC], f32, isOutput=True)
    out_gp = nc.declare_dram_parameter("out_gp", [B, O, G], f32, isOutput=True)

    with tile.TileContext(nc) as tc:
        with (
            tc.tile_pool(name="const", bufs=1) as cpool,
            tc.tile_pool(name="xtiles", bufs=P_LOC) as xpool,
            tc.tile_pool(name="acc", bufs=1) as apool,
            tc.tile_pool(name="outs", bufs=2) as opool,
            tc.tile_pool(name="psum", bufs=2, space="PSUM") as ppool,
            tc.tile_pool(name="dram", bufs=1, space="DRAM") as dpool,
        ):
            # ---- constants -------------------------------------------------
            bf16 = mybir.dt.bfloat16
            sb = {}
            for name in ("w_pgpg", "w_pgg", "w_g", "w_pgp", "w_p"):
                t = cpool.tile([128, 128], f32, tag=name)
                nc.sync.dma_start(t[:], nc.inline_tensor(consts[name], name)[:, :])
                sb[name] = t
            ident_bf = cpool.tile([128, 128], bf16, tag="ident_bf")
            nc.sync.dma_start(
                ident_bf[:],
                nc.inline_tensor(np.eye(128, dtype=ml_dtypes.bfloat16),
                                 "ident_bf")[:, :])

            # trigger the Lrelu activation-table load early, off the critical path
            dummy = cpool.tile([1, 1], f32, tag="dummy")
            nc.vector.memset(dummy[:], 0.0)
            nc.scalar.activation(dummy[:], dummy[:],
                                 mybir.ActivationFunctionType.Lrelu,
                                 bias=0.0, scale=1.0, alpha=NEG_SLOPE)
            for name in ("bias_yg", "bias_ypb"):
                t = cpool.tile([128, 1], f32, tag=name)
                nc.sync.dma_start(t[:], nc.inline_tensor(consts[name], name)[:, :])
                sb[name] = t

            # ---- small inputs ---------------------------------------------
            xg_sb = cpool.tile([128, H], f32, tag="xg_sb")
            nc.sync.dma_start(xg_sb[:], xg[:, :, :].rearrange("b c (h g) -> h b c g", h=2))
            xp_sb = cpool.tile([64, P_LOC], f32, tag="xp_sb")
            nc.sync.dma_start(xp_sb[:], xp[:, :, :].rearrange("b c p -> (b c) p"))

            # ---- accumulators ----------------------------------------------
            accp = apool.tile([128, H], f32, tag="accp")
            accp2 = apool.tile([128, H], f32, tag="accp2")
            accg = apool.tile([128, G], f32, tag="accg")
            pgc = apool.tile([128, P_LOC], f32, tag="pgc")
            outp_sb = apool.tile([128, P_LOC], f32, tag="outp_sb")

            # ---- phase 1: load x shard, pool_p chain first -----------------
            # The pool_p running max feeds the AllReduce; emit it (and the
            # collective) before the pool_g reduces so the collective fires as
            # soon as the chain ends while the reduces keep VectorE busy.
            # two independent running-max chains (even/odd p) keep VectorE
            # picking the high-priority pool work back-to-back
            xt = []
            for p in range(P_LOC):
                t = xpool.tile([128, H], f32, tag="xt")
                nc.sync.dma_start(t[:], x[:, :, p, :].rearrange(
                    "b c (h g) -> h b c g", h=2))
                xt.append(t)
                acc = accp if p % 2 == 0 else accp2
                with tc.high_priority():
                    if p < 2:
                        nc.vector.tensor_copy(acc[:], t[:, :])
                    else:
                        nc.vector.tensor_tensor(acc[:], acc[:], t[:, :],
                                                op=mybir.AluOpType.max)
            with tc.high_priority():
                nc.vector.tensor_tensor(accp[:], accp[:], accp2[:],
                                        op=mybir.AluOpType.max)

            # ---- AllReduce(max) of partial pool_p --------------------------
            bounce_in = dpool.tile([128, H], f32, tag="bin")
            bounce_out = dpool.tile([128, H], f32, tag="bout")
            nc.sync.dma_start(bounce_in[:, :], accp[:])
            if sim:
                nc.sync.dma_start(bounce_out[:], bounce_in[:])
            else:
                nc.gpsimd.collective_compute(
                    "AllReduce", mybir.AluOpType.max,
                    replica_groups=[list(range(N_CORES))],
                    ins=[bounce_in.opt()], outs=[bounce_out.opt()],
                )
            pool_sb = cpool.tile([128, H], f32, tag="pool_sb")
            nc.sync.dma_start(pool_sb[:], bounce_out[:])

            # ---- y_g = W_pgg @ pool_p + W_g @ xg (+biases via ACT) ---------
            # y_g is added to every p-row via identity matmuls; to avoid the
            # 4-cycles-per-row fp32 PE path, split it exactly into bf16
            # hi + lo parts and use two 1-cycle-per-row bf16 matmuls.
            yg_sb = cpool.tile([128, G], f32, tag="yg_sb")
            yg_hi = cpool.tile([128, G], bf16, tag="yg_hi")
            yg_lo = cpool.tile([128, G], bf16, tag="yg_lo")
            for q in range(4):
                h, sl = divmod(q, 2)
                lo, hi = 64 * h, 64 * h + 64
                gs = slice(512 * sl, 512 * sl + 512)
                ps = ppool.tile([128, 512], f32, tag="ypre", bufs=2)
                nc.tensor.matmul(ps[:], sb["w_g"][lo:hi, :], xg_sb[lo:hi, gs],
                                 start=True, stop=False)
                nc.tensor.matmul(ps[:], sb["w_pgg"][lo:hi, :], pool_sb[lo:hi, gs],
                                 start=False, stop=True)
                nc.scalar.activation(yg_sb[:, 512 * q:512 * q + 512], ps[:],
                                     mybir.ActivationFunctionType.Identity,
                                     bias=sb["bias_yg"][:, 0:1], scale=1.0)
                nc.scalar.activation(yg_hi[:, gs.start + 1024 * h:
                                           gs.start + 1024 * h + 512],
                                     yg_sb[:, 512 * q:512 * q + 512],
                                     mybir.ActivationFunctionType.Identity,
                                     bias=0.0, scale=1.0)
            nc.vector.tensor_tensor(yg_lo[:], yg_sb[:], yg_hi[:],
                                    op=mybir.AluOpType.subtract)

            # ---- pool_g reduces (overlap the collective / y_g setup) and
            # ---- y_p in quarters so the first p-rows unblock early ---------
            pgd = apool.tile([64, P_LOC], f32, tag="pgd")
            pgf = apool.tile([64, P_LOC], f32, tag="pgf")
            ypb = apool.tile([128, P_LOC], f32, tag="ypb")
            QT = P_LOC // 4
            for qt in range(4):
                qs = slice(QT * qt, QT * qt + QT)
                for p in range(QT * qt, QT * qt + QT):
                    nc.vector.reduce_max(pgc[:, p:p + 1], xt[p][:, :],
                                         axis=mybir.AxisListType.X)
                nc.sync.dma_start(pgd[:, qs], pgc[64:128, qs])
                nc.vector.tensor_tensor(pgf[:, qs], pgc[0:64, qs], pgd[:, qs],
                                        op=mybir.AluOpType.max)
                ps_yp = ppool.tile([128, QT], f32, tag="ypre", bufs=2)
                nc.tensor.matmul(ps_yp[:], sb["w_pgp"][0:64, :], pgf[:, qs],
                                 start=True, stop=False)
                nc.tensor.matmul(ps_yp[:], sb["w_p"][0:64, :], xp_sb[:, qs],
                                 start=False, stop=True)
                nc.scalar.activation(ypb[:, qs], ps_yp[:],
                                     mybir.ActivationFunctionType.Identity,
                                     bias=sb["bias_ypb"][:, 0:1], scale=1.0)

            # ---- phase 2: per p-row compute; 2 p-rows per out DMA ----------
            # PSUM: main tiles are 2 banks x bufs=3 on their own tag so the
            # PE can run main matmuls ahead of the collective; ids accumulate
            # y_g afterwards via bf16 hi/lo identity matmuls.
            for i in range(P_LOC // 2):
                ot = opool.tile([128, 2 * G], f32, tag="ot")
                for j in range(2):
                    p = 2 * i + j
                    for half in range(2):        # half: chunks (0,1) or (2,3)
                        lo, hi = 64 * half, 64 * half + 64
                        ps = ppool.tile([128, 2 * 512], f32, tag="main", bufs=3)
                        dve_add = (2 * p + half) % 5 == 2
                        for sl in range(2):      # bank within the tile
                            cs = slice(512 * sl, 512 * sl + 512)
                            nc.tensor.matmul(ps[:, cs], sb["w_pgpg"][lo:hi, :],
                                             xt[p][lo:hi, 512 * sl:512 * sl + 512],
                                             start=True, stop=dve_add)
                        ygs = slice(1024 * half, 1024 * half + 1024)
                        if dve_add:
                            # balance: some y_g adds go to VectorE (PSUM in-place)
                            nc.vector.tensor_tensor(ps[:, :], ps[:, :],
                                                    yg_sb[:, ygs],
                                                    op=mybir.AluOpType.add)
                        else:
                            for sl in range(2):
                                cs = slice(512 * sl, 512 * sl + 512)
                                ys = slice(1024 * half + 512 * sl,
                                           1024 * half + 512 * sl + 512)
                                nc.tensor.matmul(ps[:, cs], ident_bf[:],
                                                 yg_hi[:, ys],
                                                 start=False, stop=False)
                                nc.tensor.matmul(ps[:, cs], ident_bf[:],
                                                 yg_lo[:, ys],
                                                 start=False, stop=True)
                        # fused bias + LeakyRelu over this 2-bank tile
                        nc.scalar.activation(
                            ot[:, j * G + 1024 * half:j * G + 1024 * half + 1024],
                            ps[:, :],
                            mybir.ActivationFunctionType.Lrelu,
                            bias=ypb[:, p:p + 1], scale=1.0, alpha=NEG_SLOPE)
                nc.sync.dma_start(
                    out_pg[:, :, 2 * i:2 * i + 2, :].rearrange(
                        "b o p g -> (b o) (p g)"),
                    ot[:])
                nc.vector.reduce_max(
                    outp_sb[:, 2 * i:2 * i + 2],
                    ot[:].rearrange("q (p g) -> q p g", p=2),
                    axis=mybir.AxisListType.X)
                if i == 0:
                    nc.vector.tensor_copy(accg[:], ot[:, 0:G])
                else:
                    nc.vector.tensor_tensor(accg[:], accg[:], ot[:, 0:G],
                                            op=mybir.AluOpType.max)
                nc.vector.tensor_tensor(accg[:], accg[:], ot[:, G:2 * G],
                                        op=mybir.AluOpType.max)

            # ---- small outputs ---------------------------------------------
            nc.sync.dma_start(out_p[:, :, :].rearrange("b o p -> (b o) p"),
                              outp_sb[:])
            nc.sync.dma_start(out_gp[:, :, :].rearrange("b o g -> (b o) g"),
                              accg[:])

    nc.compile()
    return nc


def _get_nc(consts):
    import hashlib
    h = hashlib.sha256()
    for k in sorted(consts):
        h.update(k.encode())
        h.update(np.ascontiguousarray(consts[k]).tobytes())
    key = h.hexdigest()
    if _CACHE.get("key") != key:
        _CACHE["nc"] = _build(consts)
        _CACHE["key"] = key
    return _CACHE["nc"]


def kernel(x_pg, x_p, x_g,
           W_pg_p, b_pg_p, W_pg_g, b_pg_g, W_pg_pg, b_pg_pg,
           W_p, b_p, W_g, b_g, _trace=False):
    x_pg = np.asarray(x_pg, dtype=np.float32)
    x_p = np.asarray(x_p, dtype=np.float32)
    x_g = np.asarray(x_g, dtype=np.float32)

    def col(bias):
        return np.ascontiguousarray(
            np.tile(np.asarray(bias, np.float32), B)[:, None])

    consts = {
        "w_pgpg": _stack_weight(np.asarray(W_pg_pg, np.float32)),
        "w_pgg": _stack_weight(np.asarray(W_pg_g, np.float32)),
        "w_g": _stack_weight(np.asarray(W_g, np.float32)),
        "w_pgp": _stack_weight(np.asarray(W_pg_p, np.float32)),
        "w_p": _stack_weight(np.asarray(W_p, np.float32)),
        "bias_yg": col(np.asarray(b_pg_g, np.float32) + np.asarray(b_g, np.float32)),
        "bias_ypb": col(np.asarray(b_pg_p, np.float32) + np.asarray(b_p, np.float32)
                        + np.asarray(b_pg_pg, np.float32)),
    }
    nc = _get_nc(consts)

    x4 = x_pg[..., 0]                      # [B,C,P,G]
    xp3 = x_p[:, :, :, 0, 0]               # [B,C,P]
    xg3 = np.ascontiguousarray(x_g[:, :, 0, :, 0])  # [B,C,G]

    in_maps = []
    for i in range(N_CORES):
        sl = slice(P_LOC * i, P_LOC * (i + 1))
        in_maps.append({
            "x": np.ascontiguousarray(x4[:, :, sl, :]),
            "xp": np.ascontiguousarray(xp3[:, :, sl]),
            "xg": xg3,
        })

    res = run_bass_kernel_spmd(nc, in_maps, core_ids=list(range(N_CORES)),
                               trace=_trace)
    if _trace:
        _CACHE["last_result"] = res

    out_pg = np.concatenate([r["out_pg"] for r in res.results], axis=2)
    out_p = np.concatenate([r["out_p"] for r in res.results], axis=2)
    out_g = np.maximum.reduce([r["out_gp"] for r in res.results])

    return (out_pg[:, :, :, :, None],
            out_p[:, :, :, None, None],
            out_g[:, :, None, :, None])


# revision 41
# speedup vs baseline: 295.5381x; 1.0498x over previous
"""Trainium2 Bass kernel for nn_Pool_conv_sum_nonlin_pool (B=4,C=16,O=32,P=256,G=2048).

Strategy: shard the P axis across 8 NeuronCores (32 rows each). Per core:
  phase 1: stream the x_pg shard into SBUF (resident, 16.8MB), computing
           pool_g (max over G, local) on VectorE and the local partial of
           pool_p (max over P) split across VectorE/GpSimd.
  AllReduce(max) of the pool_p partial (512KB) across the 8 cores.
  y_p / y_g are computed on TensorE with block-diagonal stacked weights
  (partition layout (h,b,c) for inputs, (b,o) for outputs; B*O = 128).
  phase 2: per p-row, one PSUM accumulation per 512-wide g-chunk:
           identity-matmul adds y_g, the main matmul adds W_pg_pg @ x;
           ScalarE applies Lrelu(t + y_p[b,o,p] + biases) in one op;
           VectorE reduces out_p, VectorE+GpSimd keep the out_g running max;
           1MB HWDGE DMA per p-row streams out_pg back to HBM.
Host: concatenates out_pg / out_p shards, max-combines the 8 out_g partials.
"""
import sys

sys.path.insert(0, "/opt/trn_rl_repo")

import ml_dtypes
import numpy as np

import concourse.bacc as bacc
import concourse.mybir as mybir
from concourse import tile
from concourse.bass_utils import run_bass_kernel_spmd

B, C, O, P, G = 4, 16, 32, 256, 2048
N_CORES = 8
P_LOC = P // N_CORES          # 32
H = G // 2                    # 1024 (partition-half split of g)
NEG_SLOPE = 0.01
NEG_INF = -3.0e38

# NOTE: this neuronxcc build rejects compute ops on the GpSimd/Pool engine
# ("Engine Pool is invalid ... Must be DVE"), so all max/reduce work runs on
# VectorE and the fused bias+Lrelu on ScalarE.

_CACHE = {}


def _stack_weight(w: np.ndarray) -> np.ndarray:
    """[O,C] conv weight -> [128,128] lhsT with block-diag over b, duplicated
    into both partition halves (rows 64h+16b+c, cols 32b+o)."""
    s = np.zeros((64, 128), dtype=np.float32)
    blk = np.ascontiguousarray(w.T.astype(np.float32))  # [C, O]
    for b in range(B):
        s[16 * b:16 * b + 16, 32 * b:32 * b + 32] = blk
    return np.concatenate([s, s], axis=0)


def _build(consts, sim=False):
    # sim=True builds a single-core variant with the collective replaced by a
    # local DRAM copy, for TimelineSim cost-model runs (no collective support).
    nc = bacc.Bacc("TRN2", target_bir_lowering=False, debug=False,
                   num_devices=1 if sim else N_CORES)
    f32 = mybir.dt.float32

    x = nc.declare_dram_parameter("x", [B, C, P_LOC, G], f32, isOutput=False)
    xp = nc.declare_dram_parameter("xp", [B, C, P_LOC], f32, isOutput=False)
    xg = nc.declare_dram_parameter("xg", [B, C, G], f32, isOutput=False)
    out_pg = nc.declare_dram_parameter("out_pg", [B, O, P_LOC, G], f32, isOutput=True)
    out_p = nc.declare_dram_parameter("out_p", [B, O, P_LOC], f32, isOutput=True)
    out_gp = nc.declare_dram_parameter("out_gp", [B, O, G], f32, isOutput=True)

    with tile.TileContext(nc) as tc:
        with (
            tc.tile_pool(name="const", bufs=1) as cpool,
            tc.tile_pool(name="xtiles", bufs=P_LOC) as xpool,
            tc.tile_pool(name="acc", bufs=1) as apool,
            tc.tile_pool(name="outs", bufs=2) as opool,
            tc.tile_pool(name="psum", bufs=2, space="PSUM") as ppool,
            tc.tile_pool(name="dram", bufs=1, space="DRAM") as dpool,
        ):
            # ---- constants -------------------------------------------------
            bf16 = mybir.dt.bfloat16
            sb = {}
            for name in ("w_pgpg", "w_pgg", "w_g", "w_pgp", "w_p"):
                t = cpool.tile([128, 128], f32, tag=name)
                nc.sync.dma_start(t[:], nc.inline_tensor(consts[name], name)[:, :])
                sb[name] = t
            ident_bf = cpool.tile([128, 128], bf16, tag="ident_bf")
            nc.sync.dma_start(
                ident_bf[:],
                nc.inline_tensor(np.eye(128, dtype=ml_dtypes.bfloat16),
                                 "ident_bf")[:, :])

            # trigger the Lrelu activation-table load early, off the critical path
            dummy = cpool.tile([1, 1], f32, tag="dummy")
            nc.vector.memset(dummy[:], 0.0)
            nc.scalar.activation(dummy[:], dummy[:],
                                 mybir.ActivationFunctionType.Lrelu,
                                 bias=0.0, scale=1.0, alpha=NEG_SLOPE)
            for name in ("bias_yg", "bias_ypb"):
                t = cpool.tile([128, 1], f32, tag=name)
                nc.sync.dma_start(t[:], nc.inline_tensor(consts[name], name)[:, :])
                sb[name] = t

            # ---- small inputs ---------------------------------------------
            xg_sb = cpool.tile([128, H], f32, tag="xg_sb")
            nc.sync.dma_start(xg_sb[:], xg[:, :, :].rearrange("b c (h g) -> h b c g", h=2))
            xp_sb = cpool.tile([64, P_LOC], f32, tag="xp_sb")
            nc.sync.dma_start(xp_sb[:], xp[:, :, :].rearrange("b c p -> (b c) p"))

            # ---- accumulators ----------------------------------------------
            accp = apool.tile([128, H], f32, tag="accp")
            accp2 = apool.tile([128, H], f32, tag="accp2")
            # scratch destination for tensor_scalar-based max reductions
            # (2x fp32 mode vs 1x tensor_reduce; accum_out carries the max)
            scr = apool.tile([128, H], f32, tag="scr")
            outp_a = apool.tile([128, P_LOC], f32, tag="outp_a")
            outp_b = apool.tile([128, P_LOC], f32, tag="outp_b")
            accg = apool.tile([128, G], f32, tag="accg")
            pgc = apool.tile([128, P_LOC], f32, tag="pgc")
            outp_sb = apool.tile([128, P_LOC], f32, tag="outp_sb")

            # ---- phase 1: load x shard, pool_p chain first -----------------
            # The pool_p running max feeds the AllReduce; emit it (and the
            # collective) before the pool_g reduces so the collective fires as
            # soon as the chain ends while the reduces keep VectorE busy.
            # two independent running-max chains (even/odd p) keep VectorE
            # picking the high-priority pool work back-to-back
            xt = []
            for p in range(P_LOC):
                t = xpool.tile([128, H], f32, tag="xt")
                nc.sync.dma_start(t[:], x[:, :, p, :].rearrange(
                    "b c (h g) -> h b c g", h=2))
                xt.append(t)
                acc = accp if p % 2 == 0 else accp2
                with tc.high_priority():
                    if p < 2:
                        nc.vector.tensor_copy(acc[:], t[:, :])
                    else:
                        nc.vector.tensor_tensor(acc[:], acc[:], t[:, :],
                                                op=mybir.AluOpType.max)
            with tc.high_priority():
                nc.vector.tensor_tensor(accp[:], accp[:], accp2[:],
                                        op=mybir.AluOpType.max)

            # ---- AllReduce(max) of partial pool_p --------------------------
            bounce_in = dpool.tile([128, H], f32, tag="bin")
            bounce_out = dpool.tile([128, H], f32, tag="bout")
            nc.sync.dma_start(bounce_in[:, :], accp[:])
            if sim:
                nc.sync.dma_start(bounce_out[:], bounce_in[:])
            else:
                nc.gpsimd.collective_compute(
                    "AllReduce", mybir.AluOpType.max,
                    replica_groups=[list(range(N_CORES))],
                    ins=[bounce_in.opt()], outs=[bounce_out.opt()],
                )
            pool_sb = cpool.tile([128, H], f32, tag="pool_sb")
            nc.sync.dma_start(pool_sb[:], bounce_out[:])

            # ---- y_g = W_pgg @ pool_p + W_g @ xg (+biases via ACT) ---------
            # y_g is added to every p-row via identity matmuls; to avoid the
            # 4-cycles-per-row fp32 PE path, split it exactly into bf16
            # hi + lo parts and use two 1-cycle-per-row bf16 matmuls.
            yg_sb = cpool.tile([128, G], f32, tag="yg_sb")
            yg_hi = cpool.tile([128, G], bf16, tag="yg_hi")
            yg_lo = cpool.tile([128, G], bf16, tag="yg_lo")
            for q in range(4):
                h, sl = divmod(q, 2)
                lo, hi = 64 * h, 64 * h + 64
                gs = slice(512 * sl, 512 * sl + 512)
                ps = ppool.tile([128, 512], f32, tag="ypre", bufs=2)
                nc.tensor.matmul(ps[:], sb["w_g"][lo:hi, :], xg_sb[lo:hi, gs],
                                 start=True, stop=False)
                nc.tensor.matmul(ps[:], sb["w_pgg"][lo:hi, :], pool_sb[lo:hi, gs],
                                 start=False, stop=True)
                nc.scalar.activation(yg_sb[:, 512 * q:512 * q + 512], ps[:],
                                     mybir.ActivationFunctionType.Identity,
                                     bias=sb["bias_yg"][:, 0:1], scale=1.0)
                nc.scalar.activation(yg_hi[:, gs.start + 1024 * h:
                                           gs.start + 1024 * h + 512],
                                     yg_sb[:, 512 * q:512 * q + 512],
                                     mybir.ActivationFunctionType.Identity,
                                     bias=0.0, scale=1.0)
            nc.vector.tensor_tensor(yg_lo[:], yg_sb[:], yg_hi[:],
                                    op=mybir.AluOpType.subtract)

            # ---- pool_g reduces (overlap the collective / y_g setup) and
            # ---- y_p in quarters so the first p-rows unblock early ---------
            pgd = apool.tile([64, P_LOC], f32, tag="pgd")
            pgf = apool.tile([64, P_LOC], f32, tag="pgf")
            ypb = apool.tile([128, P_LOC], f32, tag="ypb")
            QT = P_LOC // 4
            for qt in range(4):
                qs = slice(QT * qt, QT * qt + QT)
                for p in range(QT * qt, QT * qt + QT):
                    nc.vector.tensor_scalar(scr[:, 0:H], xt[p][:, :], 0.0, None,
                                            op0=mybir.AluOpType.add,
                                            op1=mybir.AluOpType.max,
                                            accum_out=pgc[:, p:p + 1])
                nc.sync.dma_start(pgd[:, qs], pgc[64:128, qs])
                nc.vector.tensor_tensor(pgf[:, qs], pgc[0:64, qs], pgd[:, qs],
                                        op=mybir.AluOpType.max)
                ps_yp = ppool.tile([128, QT], f32, tag="ypre", bufs=2)
                nc.tensor.matmul(ps_yp[:], sb["w_pgp"][0:64, :], pgf[:, qs],
                                 start=True, stop=False)
                nc.tensor.matmul(ps_yp[:], sb["w_p"][0:64, :], xp_sb[:, qs],
                                 start=False, stop=True)
                nc.scalar.activation(ypb[:, qs], ps_yp[:],
                                     mybir.ActivationFunctionType.Identity,
                                     bias=sb["bias_ypb"][:, 0:1], scale=1.0)

            # ---- phase 2: per p-row compute; 2 p-rows per out DMA ----------
            # PSUM: main tiles are 2 banks x bufs=3 on their own tag so the
            # PE can run main matmuls ahead of the collective; ids accumulate
            # y_g afterwards via bf16 hi/lo identity matmuls.
            for i in range(P_LOC // 2):
                ot = opool.tile([128, 2 * G], f32, tag="ot")
                for j in range(2):
                    p = 2 * i + j
                    for half in range(2):        # half: chunks (0,1) or (2,3)
                        lo, hi = 64 * half, 64 * half + 64
                        ps = ppool.tile([128, 2 * 512], f32, tag="main", bufs=3)
                        dve_add = (2 * p + half) % 5 == 2
                        for sl in range(2):      # bank within the tile
                            cs = slice(512 * sl, 512 * sl + 512)
                            nc.tensor.matmul(ps[:, cs], sb["w_pgpg"][lo:hi, :],
                                             xt[p][lo:hi, 512 * sl:512 * sl + 512],
                                             start=True, stop=dve_add)
                        ygs = slice(1024 * half, 1024 * half + 1024)
                        if dve_add:
                            # balance: some y_g adds go to VectorE (PSUM in-place)
                            nc.vector.tensor_tensor(ps[:, :], ps[:, :],
                                                    yg_sb[:, ygs],
                                                    op=mybir.AluOpType.add)
                        else:
                            for sl in range(2):
                                cs = slice(512 * sl, 512 * sl + 512)
                                ys = slice(1024 * half + 512 * sl,
                                           1024 * half + 512 * sl + 512)
                                nc.tensor.matmul(ps[:, cs], ident_bf[:],
                                                 yg_hi[:, ys],
                                                 start=False, stop=False)
                                nc.tensor.matmul(ps[:, cs], ident_bf[:],
                                                 yg_lo[:, ys],
                                                 start=False, stop=True)
                        # fused bias + LeakyRelu over this 2-bank tile
                        nc.scalar.activation(
                            ot[:, j * G + 1024 * half:j * G + 1024 * half + 1024],
                            ps[:, :],
                            mybir.ActivationFunctionType.Lrelu,
                            bias=ypb[:, p:p + 1], scale=1.0, alpha=NEG_SLOPE)
                nc.sync.dma_start(
                    out_pg[:, :, 2 * i:2 * i + 2, :].rearrange(
                        "b o p g -> (b o) (p g)"),
                    ot[:])
                for j in range(2):
                    p = 2 * i + j
                    nc.vector.tensor_scalar(scr[:, :], ot[:, j * G:j * G + H],
                                            0.0, None,
                                            op0=mybir.AluOpType.add,
                                            op1=mybir.AluOpType.max,
                                            accum_out=outp_a[:, p:p + 1])
                    nc.vector.tensor_scalar(scr[:, :], ot[:, j * G + H:
                                                          (j + 1) * G],
                                            0.0, None,
                                            op0=mybir.AluOpType.add,
                                            op1=mybir.AluOpType.max,
                                            accum_out=outp_b[:, p:p + 1])
                if i == 0:
                    nc.vector.tensor_copy(accg[:], ot[:, 0:G])
                else:
                    nc.vector.tensor_tensor(accg[:], accg[:], ot[:, 0:G],
                                            op=mybir.AluOpType.max)
                nc.vector.tensor_tensor(accg[:], accg[:], ot[:, G:2 * G],
                                        op=mybir.AluOpType.max)

            # ---- small outputs ---------------------------------------------
            nc.vector.tensor_tensor(outp_sb[:], outp_a[:], outp_b[:],
                                    op=mybir.AluOpType.max)
            nc.sync.dma_start(out_p[:, :, :].rearrange("b o p -> (b o) p"),
                              outp_sb[:])
            nc.sync.dma_start(out_gp[:, :, :].rearrange("b o g -> (b o) g"),
                              accg[:])

    nc.compile()
    return nc


def _get_nc(consts):
    import hashlib
    h = hashlib.sha256()
    for k in sorted(consts):
        h.update(k.encode())
        h.update(np.ascontiguousarray(consts[k]).tobytes())
    key = h.hexdigest()
    if _CACHE.get("key") != key:
        _CACHE["nc"] = _build(consts)
        _CACHE["key"] = key
    return _CACHE["nc"]


def kernel(x_pg, x_p, x_g,
           W_pg_p, b_pg_p, W_pg_g, b_pg_g, W_pg_pg, b_pg_pg,
           W_p, b_p, W_g, b_g, _trace=False):
    x_pg = np.asarray(x_pg, dtype=np.float32)
    x_p = np.asarray(x_p, dtype=np.float32)
    x_g = np.asarray(x_g, dtype=np.float32)

    def col(bias):
        return np.ascontiguousarray(
            np.tile(np.asarray(bias, np.float32), B)[:, None])

    consts = {
        "w_pgpg": _stack_weight(np.asarray(W_pg_pg, np.float32)),
        "w_pgg": _stack_weight(np.asarray(W_pg_g, np.float32)),
        "w_g": _stack_weight(np.asarray(W_g, np.float32)),
        "w_pgp": _stack_weight(np.asarray(W_pg_p, np.float32)),
        "w_p": _stack_weight(np.asarray(W_p, np.float32)),
        "bias_yg": col(np.asarray(b_pg_g, np.float32) + np.asarray(b_g, np.float32)),
        "bias_ypb": col(np.asarray(b_pg_p, np.float32) + np.asarray(b_p, np.float32)
                        + np.asarray(b_pg_pg, np.float32)),
    }
    nc = _get_nc(consts)

    x4 = x_pg[..., 0]                      # [B,C,P,G]
    xp3 = x_p[:, :, :, 0, 0]               # [B,C,P]
    xg3 = np.ascontiguousarray(x_g[:, :, 0, :, 0])  # [B,C,G]

    in_maps = []
    for i in range(N_CORES):
        sl = slice(P_LOC * i, P_LOC * (i + 1))
        in_maps.append({
            "x": np.ascontiguousarray(x4[:, :, sl, :]),
            "xp": np.ascontiguousarray(xp3[:, :, sl]),
            "xg": xg3,
        })

    res = run_bass_kernel_spmd(nc, in_maps, core_ids=list(range(N_CORES)),
                               trace=_trace)
    if _trace:
        _CACHE["last_result"] = res

    out_pg = np.concatenate([r["out_pg"] for r in res.results], axis=2)
    out_p = np.concatenate([r["out_p"] for r in res.results], axis=2)
    out_g = np.maximum.reduce([r["out_gp"] for r in res.results])

    return (out_pg[:, :, :, :, None],
            out_p[:, :, :, None, None],
            out_g[:, :, None, :, None])


# revision 44
# speedup vs baseline: 303.8179x; 1.0280x over previous
"""Trainium2 Bass kernel for nn_Pool_conv_sum_nonlin_pool (B=4,C=16,O=32,P=256,G=2048).

Strategy: shard the P axis across 8 NeuronCores (32 rows each). Per core:
  phase 1: stream the x_pg shard into SBUF (resident, 16.8MB), computing
           pool_g (max over G, local) on VectorE and the local partial of
           pool_p (max over P) split across VectorE/GpSimd.
  AllReduce(max) of the pool_p partial (512KB) across the 8 cores.
  y_p / y_g are computed on TensorE with block-diagonal stacked weights
  (partition layout (h,b,c) for inputs, (b,o) for outputs; B*O = 128).
  phase 2: per p-row, one PSUM accumulation per 512-wide g-chunk:
           identity-matmul adds y_g, the main matmul adds W_pg_pg @ x;
           ScalarE applies Lrelu(t + y_p[b,o,p] + biases) in one op;
           VectorE reduces out_p, VectorE+GpSimd keep the out_g running max;
           1MB HWDGE DMA per p-row streams out_pg back to HBM.
Host: concatenates out_pg / out_p shards, max-combines the 8 out_g partials.
"""
import sys

sys.path.insert(0, "/opt/trn_rl_repo")

import ml_dtypes
import numpy as np

import concourse.bacc as bacc
import concourse.mybir as mybir
from concourse import tile
from concourse.bass_utils import run_bass_kernel_spmd

B, C, O, P, G = 4, 16, 32, 256, 2048
N_CORES = 8
P_LOC = P // N_CORES          # 32
H = G // 2                    # 1024 (partition-half split of g)
NEG_SLOPE = 0.01
NEG_INF = -3.0e38

# NOTE: this neuronxcc build rejects compute ops on the GpSimd/Pool engine
# ("Engine Pool is invalid ... Must be DVE"), so all max/reduce work runs on
# VectorE and the fused bias+Lrelu on ScalarE.

_CACHE = {}


def _stack_weight(w: np.ndarray) -> np.ndarray:
    """[O,C] conv weight -> [128,128] lhsT with block-diag over b, duplicated
    into both partition halves (rows 64h+16b+c, cols 32b+o)."""
    s = np.zeros((64, 128), dtype=np.float32)
    blk = np.ascontiguousarray(w.T.astype(np.float32))  # [C, O]
    for b in range(B):
        s[16 * b:16 * b + 16, 32 * b:32 * b + 32] = blk
    return np.concatenate([s, s], axis=0)


def _build(consts, sim=False):
    # sim=True builds a single-core variant with the collective replaced by a
    # local DRAM copy, for TimelineSim cost-model runs (no collective support).
    nc = bacc.Bacc("TRN2", target_bir_lowering=False, debug=False,
                   num_devices=1 if sim else N_CORES)
    f32 = mybir.dt.float32

    x = nc.declare_dram_parameter("x", [B, C, P_LOC, G], f32, isOutput=False)
    xp = nc.declare_dram_parameter("xp", [B, C, P_LOC], f32, isOutput=False)
    xg = nc.declare_dram_parameter("xg", [B, C, G], f32, isOutput=False)
    out_pg = nc.declare_dram_parameter("out_pg", [B, O, P_LOC, G], f32, isOutput=True)
    out_p = nc.declare_dram_parameter("out_p", [B, O, P_LOC], f32, isOutput=True)
    out_gp = nc.declare_dram_parameter("out_gp", [B, O, G], f32, isOutput=True)

    with tile.TileContext(nc) as tc:
        with (
            tc.tile_pool(name="const", bufs=1) as cpool,
            tc.tile_pool(name="xtiles", bufs=P_LOC) as xpool,
            tc.tile_pool(name="acc", bufs=1) as apool,
            tc.tile_pool(name="outs", bufs=2) as opool,
            tc.tile_pool(name="psum", bufs=2, space="PSUM") as ppool,
            tc.tile_pool(name="dram", bufs=1, space="DRAM") as dpool,
        ):
            # ---- constants -------------------------------------------------
            bf16 = mybir.dt.bfloat16
            sb = {}
            for name in ("w_pgpg", "w_pgg", "w_g", "w_pgp", "w_p"):
                t = cpool.tile([128, 128], f32, tag=name)
                nc.sync.dma_start(t[:], nc.inline_tensor(consts[name], name)[:, :])
                sb[name] = t
            ident_bf = cpool.tile([128, 128], bf16, tag="ident_bf")
            nc.sync.dma_start(
                ident_bf[:],
                nc.inline_tensor(np.eye(128, dtype=ml_dtypes.bfloat16),
                                 "ident_bf")[:, :])

            # trigger the Lrelu activation-table load early, off the critical path
            dummy = cpool.tile([1, 1], f32, tag="dummy")
            nc.vector.memset(dummy[:], 0.0)
            nc.scalar.activation(dummy[:], dummy[:],
                                 mybir.ActivationFunctionType.Lrelu,
                                 bias=0.0, scale=1.0, alpha=NEG_SLOPE)
            for name in ("bias_yg", "bias_ypb"):
                t = cpool.tile([128, 1], f32, tag=name)
                nc.sync.dma_start(t[:], nc.inline_tensor(consts[name], name)[:, :])
                sb[name] = t

            # ---- small inputs ---------------------------------------------
            xg_sb = cpool.tile([128, H], f32, tag="xg_sb")
            nc.sync.dma_start(xg_sb[:], xg[:, :, :].rearrange("b c (h g) -> h b c g", h=2))
            xp_sb = cpool.tile([64, P_LOC], f32, tag="xp_sb")
            nc.sync.dma_start(xp_sb[:], xp[:, :, :].rearrange("b c p -> (b c) p"))

            # ---- accumulators ----------------------------------------------
            accp = apool.tile([128, H], f32, tag="accp")
            accp2 = apool.tile([128, H], f32, tag="accp2")
            # scratch destination for tensor_scalar-based max reductions
            # (2x fp32 mode vs 1x tensor_reduce; accum_out carries the max)
            scr = apool.tile([128, H], f32, tag="scr")
            outp_a = apool.tile([128, P_LOC], f32, tag="outp_a")
            outp_b = apool.tile([128, P_LOC], f32, tag="outp_b")
            accg = apool.tile([128, G], f32, tag="accg")
            pgc = apool.tile([128, P_LOC], f32, tag="pgc")
            outp_sb = apool.tile([128, P_LOC], f32, tag="outp_sb")

            # ---- phase 1: load x shard, pool_p chain first -----------------
            # The pool_p running max feeds the AllReduce; emit it (and the
            # collective) before the pool_g reduces so the collective fires as
            # soon as the chain ends while the reduces keep VectorE busy.
            # two independent running-max chains (even/odd p) keep VectorE
            # picking the high-priority pool work back-to-back
            xt = []
            for p in range(P_LOC):
                t = xpool.tile([128, H], f32, tag="xt")
                nc.sync.dma_start(t[:], x[:, :, p, :].rearrange(
                    "b c (h g) -> h b c g", h=2))
                xt.append(t)
                acc = accp if p % 2 == 0 else accp2
                with tc.high_priority():
                    if p < 2:
                        nc.vector.tensor_copy(acc[:], t[:, :])
                    else:
                        nc.vector.tensor_tensor(acc[:], acc[:], t[:, :],
                                                op=mybir.AluOpType.max)
            with tc.high_priority():
                nc.vector.tensor_tensor(accp[:], accp[:], accp2[:],
                                        op=mybir.AluOpType.max)

            # ---- AllReduce(max) of partial pool_p --------------------------
            bounce_in = dpool.tile([128, H], f32, tag="bin")
            bounce_out = dpool.tile([128, H], f32, tag="bout")
            nc.sync.dma_start(bounce_in[:, :], accp[:])
            if sim:
                nc.sync.dma_start(bounce_out[:], bounce_in[:])
            else:
                nc.gpsimd.collective_compute(
                    "AllReduce", mybir.AluOpType.max,
                    replica_groups=[list(range(N_CORES))],
                    ins=[bounce_in.opt()], outs=[bounce_out.opt()],
                )
            pool_sb = cpool.tile([128, H], f32, tag="pool_sb")
            nc.sync.dma_start(pool_sb[:], bounce_out[:])

            # ---- y_g = W_pgg @ pool_p + W_g @ xg (+biases via ACT) ---------
            # y_g is added to every p-row via identity matmuls; to avoid the
            # 4-cycles-per-row fp32 PE path, split it exactly into bf16
            # hi + lo parts and use two 1-cycle-per-row bf16 matmuls.
            yg_sb = cpool.tile([128, G], f32, tag="yg_sb")
            yg_hi = cpool.tile([128, G], bf16, tag="yg_hi")
            yg_lo = cpool.tile([128, G], bf16, tag="yg_lo")
            for q in range(4):
                h, sl = divmod(q, 2)
                lo, hi = 64 * h, 64 * h + 64
                gs = slice(512 * sl, 512 * sl + 512)
                ps = ppool.tile([128, 512], f32, tag="ypre", bufs=2)
                nc.tensor.matmul(ps[:], sb["w_g"][lo:hi, :], xg_sb[lo:hi, gs],
                                 start=True, stop=False)
                nc.tensor.matmul(ps[:], sb["w_pgg"][lo:hi, :], pool_sb[lo:hi, gs],
                                 start=False, stop=True)
                nc.scalar.activation(yg_sb[:, 512 * q:512 * q + 512], ps[:],
                                     mybir.ActivationFunctionType.Identity,
                                     bias=sb["bias_yg"][:, 0:1], scale=1.0)
                nc.scalar.activation(yg_hi[:, gs.start + 1024 * h:
                                           gs.start + 1024 * h + 512],
                                     yg_sb[:, 512 * q:512 * q + 512],
                                     mybir.ActivationFunctionType.Identity,
                                     bias=0.0, scale=1.0)
            nc.vector.tensor_tensor(yg_lo[:], yg_sb[:], yg_hi[:],
                                    op=mybir.AluOpType.subtract)

            # ---- pool_g reduces (overlap the collective / y_g setup) and
            # ---- y_p in quarters so the first p-rows unblock early ---------
            pgd = apool.tile([64, P_LOC], f32, tag="pgd")
            pgf = apool.tile([64, P_LOC], f32, tag="pgf")
            ypb = apool.tile([128, P_LOC], f32, tag="ypb")
            QT = P_LOC // 4
            for qt in range(4):
                qs = slice(QT * qt, QT * qt + QT)
                for p in range(QT * qt, QT * qt + QT):
                    nc.vector.tensor_scalar(scr[:, 0:H], xt[p][:, :], 0.0, None,
                                            op0=mybir.AluOpType.add,
                                            op1=mybir.AluOpType.max,
                                            accum_out=pgc[:, p:p + 1])
                nc.sync.dma_start(pgd[:, qs], pgc[64:128, qs])
                nc.vector.tensor_tensor(pgf[:, qs], pgc[0:64, qs], pgd[:, qs],
                                        op=mybir.AluOpType.max)
                ps_yp = ppool.tile([128, QT], f32, tag="ypre", bufs=2)
                nc.tensor.matmul(ps_yp[:], sb["w_pgp"][0:64, :], pgf[:, qs],
                                 start=True, stop=False)
                nc.tensor.matmul(ps_yp[:], sb["w_p"][0:64, :], xp_sb[:, qs],
                                 start=False, stop=True)
                nc.scalar.activation(ypb[:, qs], ps_yp[:],
                                     mybir.ActivationFunctionType.Identity,
                                     bias=sb["bias_ypb"][:, 0:1], scale=1.0)

            # ---- phase 2: per p-row compute; 2 p-rows per out DMA ----------
            # PSUM: main tiles are 2 banks x bufs=3 on their own tag so the
            # PE can run main matmuls ahead of the collective; ids accumulate
            # y_g afterwards via bf16 hi/lo identity matmuls.
            for i in range(P_LOC // 2):
                ot = opool.tile([128, 2 * G], f32, tag="ot")
                for j in range(2):
                    p = 2 * i + j
                    for half in range(2):        # half: chunks (0,1) or (2,3)
                        lo, hi = 64 * half, 64 * half + 64
                        ps = ppool.tile([128, 2 * 512], f32, tag="main", bufs=3)
                        dve_add = (2 * p + half) % 2 == 0
                        for sl in range(2):      # bank within the tile
                            cs = slice(512 * sl, 512 * sl + 512)
                            nc.tensor.matmul(ps[:, cs], sb["w_pgpg"][lo:hi, :],
                                             xt[p][lo:hi, 512 * sl:512 * sl + 512],
                                             start=True, stop=dve_add)
                        ygs = slice(1024 * half, 1024 * half + 1024)
                        if dve_add:
                            # balance: some y_g adds go to VectorE (PSUM in-place)
                            nc.vector.tensor_tensor(ps[:, :], ps[:, :],
                                                    yg_sb[:, ygs],
                                                    op=mybir.AluOpType.add)
                        else:
                            for sl in range(2):
                                cs = slice(512 * sl, 512 * sl + 512)
                                ys = slice(1024 * half + 512 * sl,
                                           1024 * half + 512 * sl + 512)
                                nc.tensor.matmul(ps[:, cs], ident_bf[:],
                                                 yg_hi[:, ys],
                                                 start=False, stop=False)
                                nc.tensor.matmul(ps[:, cs], ident_bf[:],
                                                 yg_lo[:, ys],
                                                 start=False, stop=True)
                        # fused bias + LeakyRelu over this 2-bank tile
                        nc.scalar.activation(
                            ot[:, j * G + 1024 * half:j * G + 1024 * half + 1024],
                            ps[:, :],
                            mybir.ActivationFunctionType.Lrelu,
                            bias=ypb[:, p:p + 1], scale=1.0, alpha=NEG_SLOPE)
                nc.sync.dma_start(
                    out_pg[:, :, 2 * i:2 * i + 2, :].rearrange(
                        "b o p g -> (b o) (p g)"),
                    ot[:])
                for j in range(2):
                    p = 2 * i + j
                    nc.vector.tensor_scalar(scr[:, :], ot[:, j * G:j * G + H],
                                            0.0, None,
                                            op0=mybir.AluOpType.add,
                                            op1=mybir.AluOpType.max,
                                            accum_out=outp_a[:, p:p + 1])
                    nc.vector.tensor_scalar(scr[:, :], ot[:, j * G + H:
                                                          (j + 1) * G],
                                            0.0, None,
                                            op0=mybir.AluOpType.add,
                                            op1=mybir.AluOpType.max,
                                            accum_out=outp_b[:, p:p + 1])
                if i == 0:
                    nc.vector.tensor_copy(accg[:], ot[:, 0:G])
                else:
                    nc.vector.tensor_tensor(accg[:], accg[:], ot[:, 0:G],
                                            op=mybir.AluOpType.max)
                nc.vector.tensor_tensor(accg[:], accg[:], ot[:, G:2 * G],
                                        op=mybir.AluOpType.max)

            # ---- small outputs ---------------------------------------------
            nc.vector.tensor_tensor(outp_sb[:], outp_a[:], outp_b[:],
                                    op=mybir.AluOpType.max)
            nc.sync.dma_start(out_p[:, :, :].rearrange("b o p -> (b o) p"),
                              outp_sb[:])
            nc.sync.dma_start(out_gp[:, :, :].rearrange("b o g -> (b o) g"),
                              accg[:])

    nc.compile()
    return nc


def _get_nc(consts):
    import hashlib
    h = hashlib.sha256()
    for k in sorted(consts):
        h.update(k.encode())
        h.update(np.ascontiguousarray(consts[k]).tobytes())
    key = h.hexdigest()
    if _CACHE.get("key") != key:
        _CACHE["nc"] = _build(consts)
        _CACHE["key"] = key
    return _CACHE["nc"]


def kernel(x_pg, x_p, x_g,
           W_pg_p, b_pg_p, W_pg_g, b_pg_g, W_pg_pg, b_pg_pg,
           W_p, b_p, W_g, b_g, _trace=False):
    x_pg = np.asarray(x_pg, dtype=np.float32)
    x_p = np.asarray(x_p, dtype=np.float32)
    x_g = np.asarray(x_g, dtype=np.float32)

    def col(bias):
        return np.ascontiguousarray(
            np.tile(np.asarray(bias, np.float32), B)[:, None])

    consts = {
        "w_pgpg": _stack_weight(np.asarray(W_pg_pg, np.float32)),
        "w_pgg": _stack_weight(np.asarray(W_pg_g, np.float32)),
        "w_g": _stack_weight(np.asarray(W_g, np.float32)),
        "w_pgp": _stack_weight(np.asarray(W_pg_p, np.float32)),
        "w_p": _stack_weight(np.asarray(W_p, np.float32)),
        "bias_yg": col(np.asarray(b_pg_g, np.float32) + np.asarray(b_g, np.float32)),
        "bias_ypb": col(np.asarray(b_pg_p, np.float32) + np.asarray(b_p, np.float32)
                        + np.asarray(b_pg_pg, np.float32)),
    }
    nc = _get_nc(consts)

    x4 = x_pg[..., 0]                      # [B,C,P,G]
    xp3 = x_p[:, :, :, 0, 0]               # [B,C,P]
    xg3 = np.ascontiguousarray(x_g[:, :, 0, :, 0])  # [B,C,G]

    in_maps = []
    for i in range(N_CORES):
        sl = slice(P_LOC * i, P_LOC * (i + 1))
        in_maps.append({
            "x": np.ascontiguousarray(x4[:, :, sl, :]),
            "xp": np.ascontiguousarray(xp3[:, :, sl]),
            "xg": xg3,
        })

    res = run_bass_kernel_spmd(nc, in_maps, core_ids=list(range(N_CORES)),
                               trace=_trace)
    if _trace:
        _CACHE["last_result"] = res

    out_pg = np.concatenate([r["out_pg"] for r in res.results], axis=2)
    out_p = np.concatenate([r["out_p"] for r in res.results], axis=2)
    out_g = np.maximum.reduce([r["out_gp"] for r in res.results])

    return (out_pg[:, :, :, :, None],
            out_p[:, :, :, None, None],
            out_g[:, :, None, :, None])
